# revision 7
# baseline (speedup 1.0000x reference)
"""Trainium2 Bass kernel for nn_CrossAttentionWithMask.

Math (per support image n; B=1, C=64, H=W=64, L=4096):
    Q = q @ Wq.T + bq ; K = s @ Wk.T + bk ; V = s @ Wv.T + bv     [L, C]
    S = (Q @ K.T) * C**-0.5                                       [L, L]
    P = softmax(S, axis=-1)
    mask = sigmoid((max_m P - sigmoid(threshold)) * softplus(temperature))
    out = (P @ V) * mask[:, None]   -> reshaped to [C, H, W]

Sharding: 8 cores = (n in 0..3) x (half of the L query rows). Each core
computes a [2048, 4096] attention block fully independently.

Device dataflow (all in transposed [C, L] layout, which is the native
layout of the inputs):
    Ghat = [[Wk.T@Wq, Wk.T@bq], [bk@Wq, bk@bq]] * scale   (65x65, host)
    P65  = Ghat @ qhatT          (qhatT = [qT; ones])      [65, 2048]
    S^T[m, l] = shatT[:, m] . P65[:, l]                    (PE)
    es = exp(S^T)  (no max subtraction needed; |S| < ~6)   (ACT, bf16 out)
    AV: [V | ones].T @ es accumulated over m-chunks -> [65, l]; row 64 is
        the softmax denominator (PE)
    row-max of es via running tensor_max + PE transpose + free-axis reduce
    final scale = sigmoid(maxattn*tmp - thr*tmp) / denom, applied along l.
"""

import numpy as np

C = 64
L = 4096
L2 = 2048  # per-core query columns
CH = 65    # augmented (homogeneous) dim
NM = L // 128   # 32 m-chunks
N_CORES = 8

_CACHE = {}


def _build_bass():
    import concourse.bass as bass
    import concourse.mybir as mybir
    import concourse.tile as tile
    from concourse import bacc
    from concourse.masks import make_identity

    f32 = mybir.dt.float32
    bf16 = mybir.dt.bfloat16
    AF = mybir.ActivationFunctionType
    X = mybir.AxisListType.X

    nc = bacc.Bacc()
    qT = nc.declare_dram_parameter("qT", [CH, L2], f32, isOutput=False)
    sT = nc.declare_dram_parameter("sT", [CH, L], f32, isOutput=False)
    GT = nc.declare_dram_parameter("GT", [CH, CH], f32, isOutput=False)
    WV = nc.declare_dram_parameter("WV", [CH, CH], f32, isOutput=False)
    MP = nc.declare_dram_parameter("MP", [1, 2], f32, isOutput=False)
    OUT = nc.declare_dram_parameter("out", [C, L2], f32, isOutput=True)
    scr_d = nc.dram_tensor("scr_d", [L2], f32)
    scr_c = nc.dram_tensor("scr_c", [L2], f32)

    with tile.TileContext(nc) as tc:
        with (
            tc.tile_pool(name="consts", bufs=1) as consts,
            tc.tile_pool(name="big", bufs=1) as big,
            tc.tile_pool(name="es_pool", bufs=3) as es_pool,
            tc.tile_pool(name="tail", bufs=1) as tailp,
        ):
            # ---- constants (staged through DVE so matmuls wait on one sem) ----
            gts = consts.tile([CH, CH], f32)
            nc.sync.dma_start(out=gts, in_=GT[:, :])
            gt = consts.tile([CH, CH], f32)
            nc.vector.tensor_copy(gt, gts)
            wvs = consts.tile([CH, CH], f32)
            nc.sync.dma_start(out=wvs, in_=WV[:, :])
            wv = consts.tile([CH, CH], f32)
            nc.vector.tensor_copy(wv, wvs)
            mps = consts.tile([128, 2], f32)
            nc.sync.dma_start(
                out=mps,
                in_=bass.AP(tensor=MP, offset=0, ap=[[0, 128], [1, 2]]),
            )
            mpc = consts.tile([128, 2], f32)
            nc.vector.tensor_copy(mpc, mps)
            idents = consts.tile([128, 128], bf16)
            make_identity(nc, idents)
            ident = consts.tile([128, 128], bf16)
            nc.vector.tensor_copy(ident, idents)

            # ---- augmented inputs in SBUF (ones row appended host-side) ----
            qa = []
            for h in range(2):
                ts = big.tile([CH, 1024], f32, tag=f"qas{h}", name=f"qas{h}")
                nc.sync.dma_start(out=ts, in_=qT[:, h * 1024:(h + 1) * 1024])
                t = big.tile([CH, 1024], f32, tag=f"qa{h}", name=f"qa{h}")
                nc.vector.tensor_copy(t, ts)
                qa.append(t)
            sa = []
            for t4 in range(4):
                ts = big.tile([CH, 1024], f32, tag=f"sas{t4}", name=f"sas{t4}")
                nc.sync.dma_start(out=ts, in_=sT[:, t4 * 1024:(t4 + 1) * 1024])
                t = big.tile([CH, 1024], f32, tag=f"sa{t4}", name=f"sa{t4}")
                nc.vector.tensor_copy(t, ts)
                sa.append(t)

            pb = [big.tile([CH, 1024], f32, tag=f"pb{h}", name=f"pb{h}") for h in range(2)]
            vbuf = big.tile([128, NM, CH], bf16)
            rm = big.tile([128, L2], bf16)

            # ---- projections (PSUM pool closed before the main loop) ----
            # Vaug first so the later pb copies subsume vbuf's DVE ticks.
            with tc.tile_pool(name="proj_psum", bufs=2, space="PSUM") as pj:
                for m in range(NM):
                    t4, mc = divmod(m, 8)
                    vp = pj.tile([128, CH], f32, tag="vp")
                    nc.tensor.matmul(
                        vp, sa[t4][:, mc * 128:(mc + 1) * 128], wv,
                        start=True, stop=True,
                    )
                    nc.vector.tensor_copy(vbuf[:, m, :], vp)
                # P65 = Ghat @ qhatT
                for h in range(2):
                    for j in range(2):
                        pp = pj.tile([CH, 512], f32, tag="pp")
                        nc.tensor.matmul(
                            pp, gt, qa[h][:, j * 512:(j + 1) * 512],
                            start=True, stop=True,
                        )
                        nc.vector.tensor_copy(pb[h][:, j * 512:(j + 1) * 512], pp)
            nc.vector.memset(rm, 0.0)

            # ---- main loop ----
            with tc.tile_pool(name="av_psum", bufs=1, space="PSUM") as avp:
                avs = [avp.tile([CH, 512], f32, tag=f"av{lt}", name=f"av{lt}") for lt in range(4)]
                with tc.tile_pool(name="sp_psum", bufs=2, space="PSUM") as spp:
                    for m in range(NM):
                        t4, mc = divmod(m, 8)
                        lhs = sa[t4][:, mc * 128:(mc + 1) * 128]
                        for h in range(2):
                            sp = spp.tile([128, 1024], f32, tag="sp")
                            for j in range(2):
                                nc.tensor.matmul(
                                    sp[:, j * 512:(j + 1) * 512],
                                    lhs,
                                    pb[h][:, j * 512:(j + 1) * 512],
                                    start=True, stop=True,
                                )
                            es = es_pool.tile([128, 1024], bf16, tag="es")
                            nc.scalar.activation(es, sp, AF.Exp)
                            nc.vector.tensor_max(
                                rm[:, h * 1024:(h + 1) * 1024],
                                rm[:, h * 1024:(h + 1) * 1024],
                                es,
                            )
                            for j in range(2):
                                lt = h * 2 + j
                                nc.tensor.matmul(
                                    avs[lt],
                                    vbuf[:, m, :],
                                    es[:, j * 512:(j + 1) * 512],
                                    start=(m == 0), stop=(m == NM - 1),
                                )

                # ---- tail: row-max, denominator, mask, final scale ----
                # denominators (row 64 of each avs) -> scr_d -> [128, 16]
                drow = tailp.tile([1, L2], f32)
                for lt in range(4):
                    nc.vector.tensor_copy(
                        drow[:, lt * 512:(lt + 1) * 512], avs[lt][CH - 1:CH, :]
                    )
                nc.sync.dma_start(
                    out=scr_d[:].rearrange("(o l) -> o l", o=1), in_=drow
                )
                dcol = tailp.tile([128, 16], f32)
                nc.sync.dma_start(
                    out=dcol, in_=scr_d[:].rearrange("(f p) -> p f", p=128)
                )
                rd = tailp.tile([128, 16], f32)
                nc.vector.reciprocal(rd, dcol)

                # row-max of rm via PE transpose + free-axis reduce
                rx = tailp.tile([128, 16], f32)
                with tc.tile_pool(name="tp_psum", bufs=3, space="PSUM") as tpp:
                    for j in range(16):
                        tp = tpp.tile([128, 128], bf16, tag="tp")
                        nc.tensor.transpose(tp, rm[:, j * 128:(j + 1) * 128], ident)
                        nc.vector.reduce_max(rx[:, j:j + 1], tp, axis=X)

                    maxattn = tailp.tile([128, 16], f32)
                    nc.vector.tensor_mul(maxattn, rx, rd)
                    cmask = tailp.tile([128, 16], f32)
                    nc.scalar.activation(
                        cmask, maxattn, AF.Sigmoid,
                        bias=mpc[:, 1:2], scale=mpc[:, 0:1],
                    )
                    cc = tailp.tile([128, 16], f32)
                    nc.vector.tensor_mul(cc, cmask, rd)
                    nc.sync.dma_start(
                        out=scr_c[:].rearrange("(f p) -> p f", p=128), in_=cc
                    )
                    crep = tailp.tile([C, L2], f32)
                    nc.sync.dma_start(
                        out=crep,
                        in_=bass.AP(tensor=scr_c, offset=0, ap=[[0, C], [1, L2]]),
                    )
                    out_sb = tailp.tile([C, L2], f32)
                    for lt in range(4):
                        nc.vector.tensor_mul(
                            out_sb[:, lt * 512:(lt + 1) * 512],
                            avs[lt][0:C, :],
                            crep[:, lt * 512:(lt + 1) * 512],
                        )
                    nc.sync.dma_start(out=OUT[:, :], in_=out_sb)

    nc.finalize()
    return nc


def _get_bass():
    if "nc" not in _CACHE:
        _CACHE["nc"] = _build_bass()
    return _CACHE["nc"]


def _host_prep(query, support, Wq, bq, Wk, bk, Wv, bv, threshold, temperature):
    ones = np.ones((1, L), np.float32)
    q = np.concatenate([np.asarray(query, np.float32).reshape(C, L), ones], axis=0)
    s = np.concatenate(
        [np.asarray(support, np.float32).reshape(4, C, L),
         np.broadcast_to(ones, (4, 1, L))], axis=1)
    s = np.ascontiguousarray(s)
    Wq64 = np.asarray(Wq, np.float64)
    bq64 = np.asarray(bq, np.float64)
    Wk64 = np.asarray(Wk, np.float64)
    bk64 = np.asarray(bk, np.float64)
    Wv64 = np.asarray(Wv, np.float64)
    bv64 = np.asarray(bv, np.float64)
    scale = C ** -0.5

    Ghat = np.zeros((CH, CH), np.float64)
    Ghat[:C, :C] = Wk64.T @ Wq64
    Ghat[C, :C] = bk64 @ Wq64
    Ghat[:C, C] = Wk64.T @ bq64
    Ghat[C, C] = bk64 @ bq64
    Ghat *= scale
    GT = np.ascontiguousarray(Ghat.T.astype(np.float32))

    WvA = np.zeros((CH, CH), np.float64)
    WvA[:C, :C] = Wv64.T
    WvA[C, :C] = bv64
    WvA[C, C] = 1.0
    WvA = np.ascontiguousarray(WvA.astype(np.float32))

    th = float(np.asarray(threshold, np.float64))
    te = float(np.asarray(temperature, np.float64))
    thr = 1.0 / (1.0 + np.exp(-th))
    tmp = np.log1p(np.exp(-abs(te))) + max(te, 0.0)  # softplus
    MPa = np.array([[tmp, -thr * tmp]], np.float32)

    in_maps = []
    for c in range(N_CORES):
        n, half = divmod(c, 2)
        in_maps.append({
            "qT": np.ascontiguousarray(q[:, half * L2:(half + 1) * L2]),
            "sT": np.ascontiguousarray(s[n]),
            "GT": GT,
            "WV": WvA,
            "MP": MPa,
        })
    return in_maps


def kernel(query, support, support_labels, Wq, bq, Wk, bk, Wv, bv,
           threshold, temperature):
    import sys
    if "/opt/trn_rl_repo" not in sys.path:
        try:
            import concourse  # noqa: F401
        except ImportError:
            sys.path.insert(0, "/opt/trn_rl_repo")
    from concourse.bass_utils import run_bass_kernel_spmd

    in_maps = _host_prep(query, support, Wq, bq, Wk, bk, Wv, bv,
                         threshold, temperature)
    nc = _get_bass()
    res = run_bass_kernel_spmd(nc, in_maps, list(range(N_CORES))).results

    out = np.zeros((4, C, L), np.float32)
    for c in range(N_CORES):
        n, half = divmod(c, 2)
        out[n][:, half * L2:(half + 1) * L2] = res[c]["out"]
    return out.reshape(4, C, 64, 64)
